# revision 7
# baseline (speedup 1.0000x reference)
"""Trainium2 Bass kernel for nn_BilinearPairedLayer.

out[b,i,j,o] = celu(zl[b,i] @ fc_l_W^T + fc_l_b) @ W[o] @ celu(zr[b,j] @ fc_r_W^T + fc_r_b) + bb[o]

with context-3 pairing:
  zl = [x_l, shift_fwd(x_l,1), shift_bwd(x_l,1)]   (192 features)
  zr = [x_l, shift_bwd(x_r,1), shift_fwd(x_r,1)]   (faithful torch-source bug: x_l first)

Shapes: B=2, N=512, n_in=64, H=128, n_out=8 -> out [2,512,512,8] f32.

Sharding: 8 cores = (b in {0,1}) x (j-chunk in {0..3} of 128 columns).
Each core computes out[b, :, j0:j0+128, :] (as bf16; host upcasts).

Per-core dataflow (contraction dims pre-transposed onto partitions host-side).

DMA strategy (empirically tuned against NTFF packet/semaphore data):
  - ALL inputs ride the sync HWDGE ring: one merged "hot" DMA first
    (D1w fc-weight overlay | D1x x-slices+biases | D1b xlhT) whose
    first-on-ring semaphore completes fast, then Wt.
  - The scalar ring carries only the ACT table load (pulled early by a
    dummy Exp).
  - Outputs are bf16 in FOUR paired DMAs on the sync ring, each issued
    as soon as its two chunk evictions land, so the 16 shared DMA mover
    engines stream output packets while later main matmuls still run.
    Each DMA writes its own DENSE [128,1024] DRAM tensor (slices of one
    big tensor write 2KB chunks at 8KB stride: ~185 GB/s vs ~310 dense).

Compute schedule:
  0. PE warm-up matmuls on memset tiles fill the ENTIRE input-DMA wait
     (~7.3us -> ~12.5us): the PE_HAM clock gate un-throttles (1.2 ->
     2.4 GHz) only after a ~3.4us fully-busy activity window, so the
     warm stream is sized to keep the PE busy with no gaps until the
     input semaphore lands.  4 short (128-wide) warms start as early as
     possible off a tiny memset; 5 long (512-wide) warms carry to the
     DMA horizon.  2 keep-alive matmuls (lhsT = hrT pins them after v2)
     bridge the v2->main PE gap.
  1. fc biases are accumulated INTO PSUM by K=1 rank-1 matmuls
     (bias-row^T @ ones-row) that run FIRST in each accumulation group:
     celu = EXP (ACT, psum src) -> fused (-1,min 0) TS (DVE) -> max TT
     (DVE, psum operand), writing bf16.  hr first (feeds v2), hl full.
  2. v2[h, j*8+o]: per o: WT_o.T @ hrT -> psum [h, o*128+j]; TWO
     strided casts (one per og group, both on ACT after hl's EXP)
     write the j-major/o-fast INTERLEAVED bf16 layout; cast og0
     overlaps the og1 matmuls.  With this order, main-output partition
     p has o = p%8 for every chunk, so ONE shared [128,1] bias AP
     serves all evictions.
  3. TRANSPOSED main: psum[jo-block, i] = v2_c.T @ hlT, 8 matmuls N=512.
     PSUM tiles are separate (PSUM dependency tracking is coarse per
     tile); main dsts reuse retired tiles so every implicit tile-level
     WAR dep is one the schedule satisfies anyway:
       pairA: main c0/c1, then c7 (emitted AFTER evict0 -> correct WAR)
       pairB: main c2/c3;  vAB (v2 og psum): main c4/c5 (after casts)
       ps_hl: main c6 (after hl celu reads)
  4. Evictions are PER-CHUNK [128,512] fused copy+bias+bf16-cast ops
     alternating ACT (activation-with-bias) and DVE (tensor_scalar),
     ordered to chase the main matmul completions; each pair's DMA is
     issued immediately after its two evictions.

walrus's per-instruction HW structs carry at most ONE sync wait; a post-pass
splits multi-wait instructions into single-wait EventSemaphore predecessors.
"""

import numpy as np

import concourse.bass as bass
import concourse.mybir as mybir
import concourse.tile as tile
from concourse.bass_utils import run_bass_kernel_spmd

F32 = mybir.dt.float32
BF16 = mybir.dt.bfloat16

B = 2
N = 512
NIN = 64
H = 128
O = 8
JC = 128  # j-chunk per core
N_CORES = 8

# D1x packed-column offsets (bf16 elements)
_XLJ = 0              # xljT  [128]   (rows 0:64)
_XRH = 128            # xrhT  [130]   (rows 0:64)
_BRR = 258            # fc_r_b as a row on partition 0  [128]
_BLR = 386            # fc_l_b as a row on partition 0  [128]
_OBI = 514            # out-bias bb[p%8] per partition (f32 bitcast, 2 cols)
_D1XW = 516
_D1BW = 514           # xlhT (rows 64:128)

N_WARM_SHORT = 4      # 128-wide, start off the tiny wsA memset
N_WARM_LONG = 7       # 512-wide, carry PE activity to the DMA horizon


def build_nc():
    nc = bass.Bass("TRN2")

    Dh = nc.dram_tensor("Dh", [128, 3 * H + _D1XW + _D1BW], BF16,
                        kind="ExternalInput")
    Wt = nc.dram_tensor("Wt", [128, O * H], BF16, kind="ExternalInput")
    # separate dense DRAM tensors per output DMA (see docstring);
    # first and last are single chunks so the packet stream starts as
    # early as possible and the tail DMA is short.
    _OW = [512, 1024, 1024, 1024, 512]
    outs = [nc.dram_tensor(f"out{i}", [128, w], BF16,
                           kind="ExternalOutput") for i, w in enumerate(_OW)]

    with tile.TileContext(nc) as tc:
        with (
            tc.tile_pool(name="persist", bufs=1) as pp,
            tc.tile_pool(name="psum", bufs=1, space="PSUM") as psp,
        ):
            Dh_sb = pp.tile([128, 3 * H + _D1XW + _D1BW], BF16, name="Dh_sb")
            W_sb = pp.tile([128, O * H], BF16, name="W_sb")
            warm_sb = pp.tile([128, 640], BF16, name="warm_sb")
            ones_sb = pp.tile([1, N], BF16, name="ones_sb")
            hrT = pp.tile([128, JC], BF16, name="hrT")
            wsA = warm_sb[:, 0:128]
            wsB = warm_sb[:, 128:640]

            # ---- ALL inputs on the sync ring: the hot layer-1 block
            # (incl. D1b) first so it drains at full rate with a fast
            # first-on-ring sem; Wt behind it.
            nc.sync.dma_start(Dh_sb[:], Dh[:])
            nc.sync.dma_start(W_sb[:], Wt[:])

            # ---- warm-tile memsets (short lhs first so warms start
            # ASAP) + early ACT table load via a dummy Exp (dst = hrT
            # cell: only a WAW dep against the much-later celu TT).
            nc.vector.memset(wsA, 0.0)
            nc.vector.memset(wsB, 0.0)
            nc.vector.memset(ones_sb[:], 1.0)
            nc.scalar.activation(hrT[0:1, 0:1], warm_sb[0:1, 0:1],
                                 mybir.ActivationFunctionType.Exp)

            # PSUM map: separate tiles (PSUM dep tracking is coarse per
            # tile, so aliasing views would serialize unrelated ops).
            # Main matmul dsts reuse retired tiles; each reuse's implicit
            # tile-level WAR dep is one that the schedule satisfies anyway.
            ps_hr = psp.tile([128, JC], F32, name="ps_hr")
            ps_hl = psp.tile([128, N], F32, name="ps_hl")
            vAB = psp.tile([128, 1024], F32, name="vAB")
            pairA = psp.tile([128, 1024], F32, name="pairA")
            pairB = psp.tile([128, 1024], F32, name="pairB")
            ps_v0 = vAB[:, 0:512]
            ps_v1 = vAB[:, 512:1024]

            # ---- PE warm-up stream: no gaps until the input DMA lands
            for _ in range(N_WARM_SHORT):
                nc.tensor.matmul(ps_hl[:, 0:128], wsA, wsA,
                                 start=True, stop=True)
            for _ in range(N_WARM_LONG):
                nc.tensor.matmul(ps_hl[:], wsA, wsB,
                                 start=True, stop=True)

            # ---- layer 1 matmuls; K=1 bias matmul runs FIRST ----
            nc.tensor.matmul(ps_hr[:], Dh_sb[0:1, 3 * H + _BRR:3 * H + _BRR + H],
                             ones_sb[0:1, 0:JC], start=True, stop=False)
            xo = 3 * H
            rhs_r = [
                Dh_sb[:, xo + _XLJ:xo + _XLJ + JC],          # x_l[j]
                Dh_sb[:, xo + _XRH + 2:xo + _XRH + 2 + JC],  # x_r[j+1]
                Dh_sb[:, xo + _XRH:xo + _XRH + JC],          # x_r[j-1]
            ]
            for c in range(3):
                nc.tensor.matmul(
                    ps_hr[:], Dh_sb[:, c * H:(c + 1) * H],
                    rhs_r[c], start=False, stop=(c == 2),
                )

            nc.tensor.matmul(ps_hl[:], Dh_sb[0:1, 3 * H + _BLR:3 * H + _BLR + H],
                             ones_sb[0:1, 0:N], start=True, stop=False)
            xb = 3 * H + _D1XW
            rhs_l = [
                Dh_sb[:, xb + 1:xb + 1 + N],    # x_l[i]
                Dh_sb[:, xb + 0:xb + N],        # x_l[i-1] (fwd)
                Dh_sb[:, xb + 2:xb + 2 + N],    # x_l[i+1] (bwd)
            ]
            for c in range(3):
                nc.tensor.matmul(
                    ps_hl[:], Dh_sb[:, c * H:(c + 1) * H],
                    rhs_l[c], start=False, stop=(c == 2),
                )

            # ---- hr celu: e (ACT) -> TS min (DVE) -> TT max (DVE) ----
            e_r = pp.tile([128, JC], F32, name="e_r")
            nc.scalar.activation(e_r[:], ps_hr[:],
                                 mybir.ActivationFunctionType.Exp)
            nc.vector.tensor_scalar(e_r[:], e_r[:], -1.0, 0.0,
                                    mybir.AluOpType.add,
                                    mybir.AluOpType.min)
            nc.vector.tensor_tensor(hrT[:], ps_hr[:], e_r[:],
                                    mybir.AluOpType.max)

            # ---- v2 matmuls: psum [h, (o,j)] per og group ----
            for og, ps_vo in ((0, ps_v0), (1, ps_v1)):
                for ol in range(4):
                    o = og * 4 + ol
                    nc.tensor.matmul(
                        ps_vo[:, ol * JC:(ol + 1) * JC],
                        W_sb[:, o * H:(o + 1) * H], hrT[:],
                        start=True, stop=True,
                    )

            # HAM keep-alive: bridge the PE idle window between v2 and
            # main.  lhsT = hrT pins the dependency so the scheduler
            # cannot hoist these before the layer-1/v2 matmuls.
            for _ in range(2):
                nc.tensor.matmul(
                    pairA[:, 0:256], hrT, warm_sb[:, 128:384],
                    start=True, stop=True,
                )

            # ---- hl celu (two pipelined halves: each EXP hands off to
            # DVE's TS/TT while ACT EXPs the next half, so hlT lands
            # ~400ns earlier and ACT frees up sooner for the cast) ----
            hlT = pp.tile([128, N], BF16, name="hlT")
            e_l = pp.tile([128, N], F32, name="e_l")
            for h0, h1 in ((0, 256), (256, 512)):
                nc.scalar.activation(e_l[:, h0:h1], ps_hl[:, h0:h1],
                                     mybir.ActivationFunctionType.Exp)
                nc.vector.tensor_scalar(e_l[:, h0:h1], e_l[:, h0:h1],
                                        -1.0, 0.0,
                                        mybir.AluOpType.add,
                                        mybir.AluOpType.min)
                nc.vector.tensor_tensor(hlT[:, h0:h1], ps_hl[:, h0:h1],
                                        e_l[:, h0:h1], mybir.AluOpType.max)

            # ---- v2 cast to interleaved bf16 layout (col = j*8+o) as
            # ONE 4-D strided op over both og groups (0.92 col/ns vs
            # 0.75 for two split casts: the og0-overlap a split would
            # buy never materializes because ACT is EXPing hl anyway).
            v2sb = pp.tile([128, O * H], BF16, name="v2sb")
            nc.scalar.copy(
                v2sb[:].rearrange("p (j g o) -> p j g o", g=2, o=4),
                vAB[:].rearrange("p (g o j) -> p j g o", g=2, o=4))

            # ---- main (transposed): psum[jo-block, i] = v2_c.T @ hlT ----
            # chunk c partition p -> j = 16c + p//8, o = p%8
            main_dst = [
                pairA[:, 0:512], pairA[:, 512:1024],
                pairB[:, 0:512], pairB[:, 512:1024],
                vAB[:, 0:512], vAB[:, 512:1024],   # WAR: after og casts
                ps_hl[:],                          # WAR: after hl celu reads
            ]
            for c in range(7):
                nc.tensor.matmul(
                    main_dst[c], v2sb[:, c * JC:(c + 1) * JC], hlT[:],
                    start=True, stop=True,
                )

            out_sb = pp.tile([128, O * N], BF16, name="out_sb")
            ob_ap = Dh_sb[:, 3 * H + _OBI:3 * H + _OBI + 2].bitcast(F32)

            def evict(eng, src, col0):
                dst = out_sb[:, col0:col0 + 512]
                if eng is nc.scalar:
                    nc.scalar.activation(dst, src,
                                         mybir.ActivationFunctionType.Identity,
                                         bias=ob_ap, scale=1.0)
                else:
                    nc.vector.tensor_scalar_add(dst, src, ob_ap)

            # chunk -> psum src (c7 reuses pairA[0:512]; emitted AFTER
            # evict0 so the WAR dep lands correctly)
            evict(nc.scalar, pairA[:, 0:512], 0)          # e0
            nc.sync.dma_start(outs[0][:], out_sb[:, 0:512])
            nc.tensor.matmul(                             # main c7
                pairA[:, 0:512], v2sb[:, 7 * JC:8 * JC], hlT[:],
                start=True, stop=True,
            )
            evict(nc.vector, pairA[:, 512:1024], 512)     # e1
            evict(nc.scalar, pairB[:, 0:512], 1024)       # e2
            nc.sync.dma_start(outs[1][:], out_sb[:, 512:1536])
            evict(nc.vector, pairB[:, 512:1024], 1536)    # e3
            evict(nc.scalar, vAB[:, 0:512], 2048)         # e4
            nc.sync.dma_start(outs[2][:], out_sb[:, 1536:2560])
            evict(nc.vector, vAB[:, 512:1024], 2560)      # e5
            evict(nc.scalar, ps_hl[:], 3072)              # e6
            nc.sync.dma_start(outs[3][:], out_sb[:, 2560:3584])
            evict(nc.scalar, pairA[:, 0:512], 3584)       # e7
            nc.sync.dma_start(outs[4][:], out_sb[:, 3584:4096])

    _legalize_waits(nc)
    return nc


def _legalize_waits(nc):
    """walrus's per-instruction HW structs carry at most ONE sync wait.
    Split any instruction with >1 on_wait into same-engine single-wait
    EventSemaphore predecessors (engine executes them in program order)."""
    n = 0
    for bb in nc.main_func.blocks:
        insts = list(bb.instructions)
        out = []
        for ins in insts:
            si = ins.sync_info
            waits = list(si.on_wait) if si and si.on_wait else []
            if len(waits) > 1:
                for w in waits[:-1]:
                    n += 1
                    out.append(mybir.InstEventSemaphore(
                        name=f"wait-split-{n}",
                        opcode="EventSemaphore",
                        engine=ins.engine,
                        ins=[], outs=[],
                        sync_info=mybir.SyncInfo(on_wait=[w], on_update=[]),
                    ))
                si.on_wait = [waits[-1]]
            out.append(ins)
        if n:
            bb.instructions = out
    return nc


_NC_CACHE = None


def _get_nc():
    global _NC_CACHE
    if _NC_CACHE is None:
        _NC_CACHE = build_nc()
    return _NC_CACHE


def _prep_core_inputs(x_l, x_r, fc_l_W, fc_l_b, fc_r_W, fc_r_b, bilinear_W, bilinear_b):
    """Host-side sharding: build the 8 per-core input dicts."""
    import ml_dtypes

    f32 = np.float32
    bf16 = ml_dtypes.bfloat16
    x_l = np.ascontiguousarray(x_l, f32)
    x_r = np.ascontiguousarray(x_r, f32)

    # WT[g, o*H + h] = W[o, h, g]
    WT = np.ascontiguousarray(
        np.asarray(bilinear_W, f32).transpose(2, 0, 1).reshape(128, O * H)
    ).astype(bf16)

    D1w = np.zeros((128, 3 * H), bf16)
    frW = np.asarray(fc_r_W, f32)
    flW = np.asarray(fc_l_W, f32)
    for c in range(3):
        D1w[:NIN, c * H:(c + 1) * H] = frW[:, c * NIN:(c + 1) * NIN].T.astype(bf16)
        D1w[NIN:, c * H:(c + 1) * H] = flW[:, c * NIN:(c + 1) * NIN].T.astype(bf16)

    D1x_c = np.zeros((128, _D1XW), bf16)
    D1x_c[0, _BRR:_BRR + H] = np.asarray(fc_r_b, f32).astype(bf16)
    D1x_c[0, _BLR:_BLR + H] = np.asarray(fc_l_b, f32).astype(bf16)
    obi = np.asarray(bilinear_b, f32)[np.arange(128) % O]  # bb[p%8]
    D1x_c.view(np.uint16)[:, _OBI:_OBI + 2] = obi.reshape(-1, 1).view('<u2')

    # D1b per batch: xlhT rows 64:128, col t = x_l[b, t-1]
    D1bs = []
    for b in range(B):
        D1b = np.zeros((128, _D1BW), bf16)
        D1b[NIN:, 1:1 + N] = x_l[b].T.astype(bf16)
        D1bs.append(D1b)

    in_maps = []
    for core in range(N_CORES):
        b, jg = core // 4, core % 4
        j0 = jg * JC
        D1x = D1x_c.copy()
        D1x[:NIN, _XLJ:_XLJ + JC] = x_l[b, j0:j0 + JC].T.astype(bf16)
        # xrhT: col t = x_r[b, j0-1+t], zero-padded at global edges
        lo = max(j0 - 1, 0)
        hi = min(j0 + JC + 1, N)
        D1x[:NIN, _XRH + lo - (j0 - 1):_XRH + hi - (j0 - 1)] = \
            x_r[b, lo:hi].T.astype(bf16)
        in_maps.append({
            "Dh": np.concatenate([D1w, D1x, D1bs[b]], axis=1),
            "Wt": WT,
        })
    return in_maps


def _run(inputs, trace=False, **kw):
    nc = _get_nc()
    in_maps = _prep_core_inputs(**inputs)
    res = run_bass_kernel_spmd(
        nc, in_maps, core_ids=list(range(N_CORES)), trace=trace, **kw)
    out = np.empty((B, N, N, O), np.float32)
    for core in range(N_CORES):
        b, jg = core // 4, core % 4
        j0 = jg * JC
        # device out: [p = jr*8+o, c*512 + i] -> out[i, 16c+jr, o]
        r = res.results[core]
        arr = np.concatenate(
            [np.asarray(r[f"out{i}"]) for i in range(5)],
            axis=1).astype(np.float32)
        arr = arr.reshape(16, 8, 8, N)          # [jr, o, c, i]
        out[b, :, j0:j0 + JC, :] = \
            arr.transpose(3, 2, 0, 1).reshape(N, JC, O)
    return out, res


def kernel(**inputs):
    out, _ = _run(inputs, trace=False)
    return out


# revision 9
# speedup vs baseline: 1.0525x; 1.0525x over previous
"""Trainium2 Bass kernel for nn_BilinearPairedLayer.

out[b,i,j,o] = celu(zl[b,i] @ fc_l_W^T + fc_l_b) @ W[o] @ celu(zr[b,j] @ fc_r_W^T + fc_r_b) + bb[o]

with context-3 pairing:
  zl = [x_l, shift_fwd(x_l,1), shift_bwd(x_l,1)]   (192 features)
  zr = [x_l, shift_bwd(x_r,1), shift_fwd(x_r,1)]   (faithful torch-source bug: x_l first)

Shapes: B=2, N=512, n_in=64, H=128, n_out=8 -> out [2,512,512,8] f32.

Sharding: 8 cores = (b in {0,1}) x (j-chunk in {0..3} of 128 columns).
Each core computes out[b, :, j0:j0+128, :] (as bf16; host upcasts).

Per-core dataflow (contraction dims pre-transposed onto partitions host-side).

DMA strategy (empirically tuned against NTFF packet/semaphore data):
  - ALL inputs ride the sync HWDGE ring: one merged "hot" DMA first
    (D1w fc-weight overlay | D1x x-slices+biases | D1b xlhT) whose
    first-on-ring semaphore completes fast, then Wt.
  - The scalar ring carries only the ACT table load (pulled early by a
    dummy Exp).
  - Outputs are bf16 in FOUR paired DMAs on the sync ring, each issued
    as soon as its two chunk evictions land, so the 16 shared DMA mover
    engines stream output packets while later main matmuls still run.
    Each DMA writes its own DENSE [128,1024] DRAM tensor (slices of one
    big tensor write 2KB chunks at 8KB stride: ~185 GB/s vs ~310 dense).

Compute schedule:
  0. PE warm-up matmuls on memset tiles fill the ENTIRE input-DMA wait
     (~7.3us -> ~12.5us): the PE_HAM clock gate un-throttles (1.2 ->
     2.4 GHz) only after a ~3.4us fully-busy activity window, so the
     warm stream is sized to keep the PE busy with no gaps until the
     input semaphore lands.  4 short (128-wide) warms start as early as
     possible off a tiny memset; 5 long (512-wide) warms carry to the
     DMA horizon.  2 keep-alive matmuls (lhsT = hrT pins them after v2)
     bridge the v2->main PE gap.
  1. fc biases are accumulated INTO PSUM by K=1 rank-1 matmuls
     (bias-row^T @ ones-row) that run FIRST in each accumulation group:
     celu = EXP (ACT, psum src) -> fused (-1,min 0) TS (DVE) -> max TT
     (DVE, psum operand), writing bf16.  hr first (feeds v2), hl full.
  2. v2[h, j*8+o]: per o: WT_o.T @ hrT -> psum [h, o*128+j]; TWO
     strided casts (one per og group, both on ACT after hl's EXP)
     write the j-major/o-fast INTERLEAVED bf16 layout; cast og0
     overlaps the og1 matmuls.  With this order, main-output partition
     p has o = p%8 for every chunk, so ONE shared [128,1] bias AP
     serves all evictions.
  3. TRANSPOSED main: psum[jo-block, i] = v2_c.T @ hlT, 8 matmuls N=512.
     PSUM tiles are separate (PSUM dependency tracking is coarse per
     tile); main dsts reuse retired tiles so every implicit tile-level
     WAR dep is one the schedule satisfies anyway:
       pairA: main c0/c1, then c7 (emitted AFTER evict0 -> correct WAR)
       pairB: main c2/c3;  vAB (v2 og psum): main c4/c5 (after casts)
       ps_hl: main c6 (after hl celu reads)
  4. Evictions are PER-CHUNK [128,512] fused copy+bias+bf16-cast ops
     alternating ACT (activation-with-bias) and DVE (tensor_scalar),
     ordered to chase the main matmul completions; each pair's DMA is
     issued immediately after its two evictions.

walrus's per-instruction HW structs carry at most ONE sync wait; a post-pass
splits multi-wait instructions into single-wait EventSemaphore predecessors.
"""

import numpy as np

import concourse.bass as bass
import concourse.mybir as mybir
import concourse.tile as tile
from concourse.bass_utils import run_bass_kernel_spmd

F32 = mybir.dt.float32
BF16 = mybir.dt.bfloat16

B = 2
N = 512
NIN = 64
H = 128
O = 8
JC = 128  # j-chunk per core
N_CORES = 8

# D1x packed-column offsets (bf16 elements)
_XLJ = 0              # xljT  [128]   (rows 0:64)
_XRH = 128            # xrhT  [130]   (rows 0:64)
_BRR = 258            # fc_r_b as a row on partition 0  [128]
_BLR = 386            # fc_l_b as a row on partition 0  [128]
_OBI = 514            # out-bias bb[p%8] per partition (f32 bitcast, 2 cols)
_D1XW = 516
_D1BW = 514           # xlhT (rows 64:128)

N_WARM_SHORT = 4      # 128-wide, start off the tiny wsA memset
N_WARM_LONG = 7       # 512-wide, carry PE activity to the DMA horizon


def build_nc():
    nc = bass.Bass("TRN2")

    Dh = nc.dram_tensor("Dh", [128, 3 * H + _D1XW + _D1BW], BF16,
                        kind="ExternalInput")
    Wt = nc.dram_tensor("Wt", [128, O * H], BF16, kind="ExternalInput")
    # separate dense DRAM tensors per output DMA (see docstring);
    # first and last are single chunks so the packet stream starts as
    # early as possible and the tail DMA is short.
    _OW = [512, 1024, 1024, 1024, 512]
    outs = [nc.dram_tensor(f"out{i}", [128, w], BF16,
                           kind="ExternalOutput") for i, w in enumerate(_OW)]

    with tile.TileContext(nc) as tc:
        with (
            tc.tile_pool(name="persist", bufs=1) as pp,
            tc.tile_pool(name="psum", bufs=1, space="PSUM") as psp,
        ):
            Dh_sb = pp.tile([128, 3 * H + _D1XW + _D1BW], BF16, name="Dh_sb")
            W_sb = pp.tile([128, O * H], BF16, name="W_sb")
            warm_sb = pp.tile([128, 640], BF16, name="warm_sb")
            ones_sb = pp.tile([1, N], BF16, name="ones_sb")
            hrT = pp.tile([128, JC], BF16, name="hrT")
            wsA = warm_sb[:, 0:128]
            wsB = warm_sb[:, 128:640]

            # ---- ALL inputs on the sync ring: the hot layer-1 block
            # (incl. D1b) first so it drains at full rate with a fast
            # first-on-ring sem; Wt behind it.
            nc.sync.dma_start(Dh_sb[:], Dh[:])
            nc.sync.dma_start(W_sb[:], Wt[:])

            # ---- warm-tile memsets (short lhs first so warms start
            # ASAP) + early ACT table load via a dummy Exp (dst = hrT
            # cell: only a WAW dep against the much-later celu TT).
            nc.vector.memset(wsA, 0.0)
            nc.vector.memset(wsB, 0.0)
            nc.vector.memset(ones_sb[:], 1.0)
            nc.scalar.activation(hrT[0:1, 0:1], warm_sb[0:1, 0:1],
                                 mybir.ActivationFunctionType.Exp)

            # PSUM map: separate tiles (PSUM dep tracking is coarse per
            # tile, so aliasing views would serialize unrelated ops).
            # Main matmul dsts reuse retired tiles; each reuse's implicit
            # tile-level WAR dep is one that the schedule satisfies anyway.
            ps_hr = psp.tile([128, JC], F32, name="ps_hr")
            ps_hl = psp.tile([128, N], F32, name="ps_hl")
            vAB = psp.tile([128, 1024], F32, name="vAB")
            pairA = psp.tile([128, 1024], F32, name="pairA")
            pairB = psp.tile([128, 1024], F32, name="pairB")
            ps_v0 = vAB[:, 0:512]
            ps_v1 = vAB[:, 512:1024]

            # ---- PE warm-up stream: no gaps until the input DMA lands
            for _ in range(N_WARM_SHORT):
                nc.tensor.matmul(ps_hl[:, 0:128], wsA, wsA,
                                 start=True, stop=True)
            for _ in range(N_WARM_LONG):
                nc.tensor.matmul(ps_hl[:], wsA, wsB,
                                 start=True, stop=True)

            # ---- layer 1 matmuls; K=1 bias matmul runs FIRST ----
            nc.tensor.matmul(ps_hr[:], Dh_sb[0:1, 3 * H + _BRR:3 * H + _BRR + H],
                             ones_sb[0:1, 0:JC], start=True, stop=False)
            xo = 3 * H
            rhs_r = [
                Dh_sb[:, xo + _XLJ:xo + _XLJ + JC],          # x_l[j]
                Dh_sb[:, xo + _XRH + 2:xo + _XRH + 2 + JC],  # x_r[j+1]
                Dh_sb[:, xo + _XRH:xo + _XRH + JC],          # x_r[j-1]
            ]
            for c in range(3):
                nc.tensor.matmul(
                    ps_hr[:], Dh_sb[:, c * H:(c + 1) * H],
                    rhs_r[c], start=False, stop=(c == 2),
                )

            nc.tensor.matmul(ps_hl[:], Dh_sb[0:1, 3 * H + _BLR:3 * H + _BLR + H],
                             ones_sb[0:1, 0:N], start=True, stop=False)
            xb = 3 * H + _D1XW
            rhs_l = [
                Dh_sb[:, xb + 1:xb + 1 + N],    # x_l[i]
                Dh_sb[:, xb + 0:xb + N],        # x_l[i-1] (fwd)
                Dh_sb[:, xb + 2:xb + 2 + N],    # x_l[i+1] (bwd)
            ]
            for c in range(3):
                nc.tensor.matmul(
                    ps_hl[:], Dh_sb[:, c * H:(c + 1) * H],
                    rhs_l[c], start=False, stop=(c == 2),
                )

            # ---- hr celu: e (ACT) -> TS min (DVE) -> TT max (DVE) ----
            e_r = pp.tile([128, JC], F32, name="e_r")
            nc.scalar.activation(e_r[:], ps_hr[:],
                                 mybir.ActivationFunctionType.Exp)
            nc.vector.tensor_scalar(e_r[:], e_r[:], -1.0, 0.0,
                                    mybir.AluOpType.add,
                                    mybir.AluOpType.min)
            nc.vector.tensor_tensor(hrT[:], ps_hr[:], e_r[:],
                                    mybir.AluOpType.max)

            # ---- v2 matmuls: psum [h, (o,j)] per og group ----
            for og, ps_vo in ((0, ps_v0), (1, ps_v1)):
                for ol in range(4):
                    o = og * 4 + ol
                    nc.tensor.matmul(
                        ps_vo[:, ol * JC:(ol + 1) * JC],
                        W_sb[:, o * H:(o + 1) * H], hrT[:],
                        start=True, stop=True,
                    )

            # HAM keep-alive: bridge the PE idle window between v2 and
            # main.  lhsT = hrT pins the dependency so the scheduler
            # cannot hoist these before the layer-1/v2 matmuls.
            for _ in range(2):
                nc.tensor.matmul(
                    pairA[:, 0:256], hrT, warm_sb[:, 128:384],
                    start=True, stop=True,
                )

            # ---- hl celu (full width; splitting into halves loses:
            # the halves share tiles, so coarse per-tile dep tracking
            # serializes EXP-half-1 behind half-0's TS/TT, and the
            # cast gets scheduled between the EXPs) ----
            hlT = pp.tile([128, N], BF16, name="hlT")
            e_l = pp.tile([128, N], F32, name="e_l")
            nc.scalar.activation(e_l[:], ps_hl[:],
                                 mybir.ActivationFunctionType.Exp)
            nc.vector.tensor_scalar(e_l[:], e_l[:], -1.0, 0.0,
                                    mybir.AluOpType.add,
                                    mybir.AluOpType.min)
            nc.vector.tensor_tensor(hlT[:], ps_hl[:],
                                    e_l[:], mybir.AluOpType.max)

            # ---- v2 cast to interleaved bf16 layout (col = j*8+o) as
            # ONE 4-D strided op over both og groups (0.92 col/ns vs
            # 0.75 for two split casts: the og0-overlap a split would
            # buy never materializes because ACT is EXPing hl anyway).
            v2sb = pp.tile([128, O * H], BF16, name="v2sb")
            nc.scalar.copy(
                v2sb[:].rearrange("p (j g o) -> p j g o", g=2, o=4),
                vAB[:].rearrange("p (g o j) -> p j g o", g=2, o=4))

            # ---- main (transposed): psum[jo-block, i] = v2_c.T @ hlT ----
            # chunk c partition p -> j = 16c + p//8, o = p%8
            main_dst = [
                pairA[:, 0:512], pairA[:, 512:1024],
                pairB[:, 0:512], pairB[:, 512:1024],
                vAB[:, 0:512], vAB[:, 512:1024],   # WAR: after og casts
                ps_hl[:],                          # WAR: after hl celu reads
            ]
            for c in range(7):
                nc.tensor.matmul(
                    main_dst[c], v2sb[:, c * JC:(c + 1) * JC], hlT[:],
                    start=True, stop=True,
                )

            out_sb = pp.tile([128, O * N], BF16, name="out_sb")
            ob_ap = Dh_sb[:, 3 * H + _OBI:3 * H + _OBI + 2].bitcast(F32)

            def evict(eng, src, col0):
                dst = out_sb[:, col0:col0 + 512]
                if eng is nc.scalar:
                    nc.scalar.activation(dst, src,
                                         mybir.ActivationFunctionType.Identity,
                                         bias=ob_ap, scale=1.0)
                else:
                    nc.vector.tensor_scalar_add(dst, src, ob_ap)

            # chunk -> psum src (c7 reuses pairA[0:512]; emitted AFTER
            # evict0 so the WAR dep lands correctly)
            evict(nc.scalar, pairA[:, 0:512], 0)          # e0
            nc.sync.dma_start(outs[0][:], out_sb[:, 0:512])
            evict(nc.vector, pairA[:, 512:1024], 512)     # e1
            # c7 emitted AFTER e1: PSUM dep tracking is coarse per
            # tile, so an e1 emitted later would wait c7's write to
            # the pairA tile even though the column ranges differ.
            nc.tensor.matmul(                             # main c7
                pairA[:, 0:512], v2sb[:, 7 * JC:8 * JC], hlT[:],
                start=True, stop=True,
            )
            evict(nc.scalar, pairB[:, 0:512], 1024)       # e2
            nc.sync.dma_start(outs[1][:], out_sb[:, 512:1536])
            evict(nc.vector, pairB[:, 512:1024], 1536)    # e3
            evict(nc.scalar, vAB[:, 0:512], 2048)         # e4
            nc.sync.dma_start(outs[2][:], out_sb[:, 1536:2560])
            evict(nc.vector, vAB[:, 512:1024], 2560)      # e5
            evict(nc.scalar, ps_hl[:], 3072)              # e6
            nc.sync.dma_start(outs[3][:], out_sb[:, 2560:3584])
            evict(nc.scalar, pairA[:, 0:512], 3584)       # e7
            nc.sync.dma_start(outs[4][:], out_sb[:, 3584:4096])

    _legalize_waits(nc)
    return nc


def _legalize_waits(nc):
    """walrus's per-instruction HW structs carry at most ONE sync wait.
    Split any instruction with >1 on_wait into same-engine single-wait
    EventSemaphore predecessors (engine executes them in program order)."""
    n = 0
    for bb in nc.main_func.blocks:
        insts = list(bb.instructions)
        out = []
        for ins in insts:
            si = ins.sync_info
            waits = list(si.on_wait) if si and si.on_wait else []
            if len(waits) > 1:
                for w in waits[:-1]:
                    n += 1
                    out.append(mybir.InstEventSemaphore(
                        name=f"wait-split-{n}",
                        opcode="EventSemaphore",
                        engine=ins.engine,
                        ins=[], outs=[],
                        sync_info=mybir.SyncInfo(on_wait=[w], on_update=[]),
                    ))
                si.on_wait = [waits[-1]]
            out.append(ins)
        if n:
            bb.instructions = out
    return nc


_NC_CACHE = None


def _get_nc():
    global _NC_CACHE
    if _NC_CACHE is None:
        _NC_CACHE = build_nc()
    return _NC_CACHE


def _prep_core_inputs(x_l, x_r, fc_l_W, fc_l_b, fc_r_W, fc_r_b, bilinear_W, bilinear_b):
    """Host-side sharding: build the 8 per-core input dicts."""
    import ml_dtypes

    f32 = np.float32
    bf16 = ml_dtypes.bfloat16
    x_l = np.ascontiguousarray(x_l, f32)
    x_r = np.ascontiguousarray(x_r, f32)

    # WT[g, o*H + h] = W[o, h, g]
    WT = np.ascontiguousarray(
        np.asarray(bilinear_W, f32).transpose(2, 0, 1).reshape(128, O * H)
    ).astype(bf16)

    D1w = np.zeros((128, 3 * H), bf16)
    frW = np.asarray(fc_r_W, f32)
    flW = np.asarray(fc_l_W, f32)
    for c in range(3):
        D1w[:NIN, c * H:(c + 1) * H] = frW[:, c * NIN:(c + 1) * NIN].T.astype(bf16)
        D1w[NIN:, c * H:(c + 1) * H] = flW[:, c * NIN:(c + 1) * NIN].T.astype(bf16)

    D1x_c = np.zeros((128, _D1XW), bf16)
    D1x_c[0, _BRR:_BRR + H] = np.asarray(fc_r_b, f32).astype(bf16)
    D1x_c[0, _BLR:_BLR + H] = np.asarray(fc_l_b, f32).astype(bf16)
    obi = np.asarray(bilinear_b, f32)[np.arange(128) % O]  # bb[p%8]
    D1x_c.view(np.uint16)[:, _OBI:_OBI + 2] = obi.reshape(-1, 1).view('<u2')

    # D1b per batch: xlhT rows 64:128, col t = x_l[b, t-1]
    D1bs = []
    for b in range(B):
        D1b = np.zeros((128, _D1BW), bf16)
        D1b[NIN:, 1:1 + N] = x_l[b].T.astype(bf16)
        D1bs.append(D1b)

    in_maps = []
    for core in range(N_CORES):
        b, jg = core // 4, core % 4
        j0 = jg * JC
        D1x = D1x_c.copy()
        D1x[:NIN, _XLJ:_XLJ + JC] = x_l[b, j0:j0 + JC].T.astype(bf16)
        # xrhT: col t = x_r[b, j0-1+t], zero-padded at global edges
        lo = max(j0 - 1, 0)
        hi = min(j0 + JC + 1, N)
        D1x[:NIN, _XRH + lo - (j0 - 1):_XRH + hi - (j0 - 1)] = \
            x_r[b, lo:hi].T.astype(bf16)
        in_maps.append({
            "Dh": np.concatenate([D1w, D1x, D1bs[b]], axis=1),
            "Wt": WT,
        })
    return in_maps


def _run(inputs, trace=False, **kw):
    nc = _get_nc()
    in_maps = _prep_core_inputs(**inputs)
    res = run_bass_kernel_spmd(
        nc, in_maps, core_ids=list(range(N_CORES)), trace=trace, **kw)
    out = np.empty((B, N, N, O), np.float32)
    for core in range(N_CORES):
        b, jg = core // 4, core % 4
        j0 = jg * JC
        # device out: [p = jr*8+o, c*512 + i] -> out[i, 16c+jr, o]
        r = res.results[core]
        arr = np.concatenate(
            [np.asarray(r[f"out{i}"]) for i in range(5)],
            axis=1).astype(np.float32)
        arr = arr.reshape(16, 8, 8, N)          # [jr, o, c, i]
        out[b, :, j0:j0 + JC, :] = \
            arr.transpose(3, 2, 0, 1).reshape(N, JC, O)
    return out, res


def kernel(**inputs):
    out, _ = _run(inputs, trace=False)
    return out


# revision 10
# speedup vs baseline: 1.0823x; 1.0283x over previous
"""Trainium2 Bass kernel for nn_BilinearPairedLayer.

out[b,i,j,o] = celu(zl[b,i] @ fc_l_W^T + fc_l_b) @ W[o] @ celu(zr[b,j] @ fc_r_W^T + fc_r_b) + bb[o]

with context-3 pairing:
  zl = [x_l, shift_fwd(x_l,1), shift_bwd(x_l,1)]   (192 features)
  zr = [x_l, shift_bwd(x_r,1), shift_fwd(x_r,1)]   (faithful torch-source bug: x_l first)

Shapes: B=2, N=512, n_in=64, H=128, n_out=8 -> out [2,512,512,8] f32.

Sharding: 8 cores = (b in {0,1}) x (j-chunk in {0..3} of 128 columns).
Each core computes out[b, :, j0:j0+128, :] (as bf16; host upcasts).

Per-core dataflow (contraction dims pre-transposed onto partitions host-side).

DMA strategy (empirically tuned against NTFF packet/semaphore data):
  - ALL inputs ride the sync HWDGE ring: one merged "hot" DMA first
    (D1w fc-weight overlay | D1x x-slices+biases | D1b xlhT) whose
    first-on-ring semaphore completes fast, then Wt.
  - The scalar ring carries only the ACT table load (pulled early by a
    dummy Exp).
  - Outputs are bf16 in FOUR paired DMAs on the sync ring, each issued
    as soon as its two chunk evictions land, so the 16 shared DMA mover
    engines stream output packets while later main matmuls still run.
    Each DMA writes its own DENSE [128,1024] DRAM tensor (slices of one
    big tensor write 2KB chunks at 8KB stride: ~185 GB/s vs ~310 dense).

Compute schedule:
  0. PE warm-up matmuls on memset tiles fill the ENTIRE input-DMA wait
     (~7.3us -> ~12.5us): the PE_HAM clock gate un-throttles (1.2 ->
     2.4 GHz) only after a ~3.4us fully-busy activity window, so the
     warm stream is sized to keep the PE busy with no gaps until the
     input semaphore lands.  4 short (128-wide) warms start as early as
     possible off a tiny memset; 5 long (512-wide) warms carry to the
     DMA horizon.  2 keep-alive matmuls (lhsT = hrT pins them after v2)
     bridge the v2->main PE gap.
  1. fc biases are accumulated INTO PSUM by K=1 rank-1 matmuls
     (bias-row^T @ ones-row) that run FIRST in each accumulation group:
     celu = EXP (ACT, psum src) -> fused (-1,min 0) TS (DVE) -> max TT
     (DVE, psum operand), writing bf16.  hr first (feeds v2), hl full.
  2. v2[h, j*8+o]: per o: WT_o.T @ hrT -> psum [h, o*128+j]; TWO
     strided casts (one per og group, both on ACT after hl's EXP)
     write the j-major/o-fast INTERLEAVED bf16 layout; cast og0
     overlaps the og1 matmuls.  With this order, main-output partition
     p has o = p%8 for every chunk, so ONE shared [128,1] bias AP
     serves all evictions.
  3. TRANSPOSED main: psum[jo-block, i] = v2_c.T @ hlT, 8 matmuls N=512.
     PSUM tiles are separate (PSUM dependency tracking is coarse per
     tile); main dsts reuse retired tiles so every implicit tile-level
     WAR dep is one the schedule satisfies anyway:
       pairA: main c0/c1, then c7 (emitted AFTER evict0 -> correct WAR)
       pairB: main c2/c3;  vAB (v2 og psum): main c4/c5 (after casts)
       ps_hl: main c6 (after hl celu reads)
  4. Evictions are PER-CHUNK [128,512] fused copy+bias+bf16-cast ops
     alternating ACT (activation-with-bias) and DVE (tensor_scalar),
     ordered to chase the main matmul completions; each pair's DMA is
     issued immediately after its two evictions.

walrus's per-instruction HW structs carry at most ONE sync wait; a post-pass
splits multi-wait instructions into single-wait EventSemaphore predecessors.
"""

import numpy as np

import concourse.bass as bass
import concourse.mybir as mybir
import concourse.tile as tile
from concourse.bass_utils import run_bass_kernel_spmd

F32 = mybir.dt.float32
BF16 = mybir.dt.bfloat16

B = 2
N = 512
NIN = 64
H = 128
O = 8
JC = 128  # j-chunk per core
N_CORES = 8

# D1x packed-column offsets (bf16 elements)
_XLJ = 0              # xljT  [128]   (rows 0:64)
_XRH = 128            # xrhT  [130]   (rows 0:64)
_BRR = 258            # fc_r_b as a row on partition 0  [128]
_BLR = 386            # fc_l_b as a row on partition 0  [128]
_OBI = 514            # out-bias bb[p%8] per partition (f32 bitcast, 2 cols)
_D1XW = 516
_D1BW = 514           # xlhT (rows 64:128)

N_WARM_SHORT = 4      # 128-wide, start off the tiny wsA memset
N_WARM_LONG = 7       # 512-wide, carry PE activity to the DMA horizon


def build_nc():
    nc = bass.Bass("TRN2")

    Dh = nc.dram_tensor("Dh", [128, 3 * H + _D1XW + _D1BW], BF16,
                        kind="ExternalInput")
    Wt = nc.dram_tensor("Wt", [128, O * H], BF16, kind="ExternalInput")
    # separate dense DRAM tensors per output DMA (see docstring);
    # first and last are single chunks so the packet stream starts as
    # early as possible and the tail DMA is short.
    _OW = [512, 1024, 1024, 1024, 512]
    outs = [nc.dram_tensor(f"out{i}", [128, w], BF16,
                           kind="ExternalOutput") for i, w in enumerate(_OW)]

    with tile.TileContext(nc) as tc:
        with (
            tc.tile_pool(name="persist", bufs=1) as pp,
            tc.tile_pool(name="psum", bufs=1, space="PSUM") as psp,
        ):
            Dh_sb = pp.tile([128, 3 * H + _D1XW + _D1BW], BF16, name="Dh_sb")
            W_sb = pp.tile([128, O * H], BF16, name="W_sb")
            warm_sb = pp.tile([128, 640], BF16, name="warm_sb")
            ones_sb = pp.tile([1, N], BF16, name="ones_sb")
            hrT = pp.tile([128, JC], BF16, name="hrT")
            wsA = warm_sb[:, 0:128]
            wsB = warm_sb[:, 128:640]

            # ---- ALL inputs on the sync ring: the hot layer-1 block
            # (incl. D1b) first so it drains at full rate with a fast
            # first-on-ring sem; Wt behind it.
            nc.sync.dma_start(Dh_sb[:], Dh[:])
            nc.sync.dma_start(W_sb[:], Wt[:])

            # ---- warm-tile memsets (short lhs first so warms start
            # ASAP) + early ACT table load via a dummy Exp (dst = hrT
            # cell: only a WAW dep against the much-later celu TT).
            nc.vector.memset(wsA, 0.0)
            nc.vector.memset(wsB, 0.0)
            nc.vector.memset(ones_sb[:], 1.0)
            nc.scalar.activation(hrT[0:1, 0:1], warm_sb[0:1, 0:1],
                                 mybir.ActivationFunctionType.Exp)

            # PSUM map (4096 f32 cols, exactly full).  Dep tracking is
            # COARSE PER TILE: a reader waits the tile's LAST
            # earlier-emitted write, a writer waits ALL earlier reads
            # of the tile.  The layout below is chosen so every
            # implicit tile-level dep is one the schedule satisfies:
            #   S0  [512]: warm-ups | hr layer-1 [0:128] | keep-alives
            #              | main c0.  (hr-celu reads finish long
            #              before c0; e0 evicts after c0.)
            #   P12 [1024]: v2 og0 [0:512] | mains c1, c2.  (cast0
            #              reads og0; c1/c2 overwrite after the cast.)
            #   P34 [1024]: v2 og1 [0:512] | mains c3, c4.
            #   P56 [1024]: hl G0 [0:256] | mains c5, c6.
            #   S7  [512]: hl G1 [0:256] | main c7.  (hl split across
            #              two tiles keeps EXP(G0) independent of G1's
            #              matmuls; c7 has NO eviction entanglement.)
            S0 = psp.tile([128, 512], F32, name="S0")
            P12 = psp.tile([128, 1024], F32, name="P12")
            P34 = psp.tile([128, 1024], F32, name="P34")
            P56 = psp.tile([128, 1024], F32, name="P56")
            S7 = psp.tile([128, 512], F32, name="S7")
            ps_hr = S0[:, 0:128]
            ps_v0 = P12[:, 0:512]
            ps_v1 = P34[:, 0:512]
            ps_hl0 = P56[:, 0:256]
            ps_hl1 = S7[:, 0:256]

            # ---- PE warm-up stream: no gaps until the input DMA lands
            for _ in range(N_WARM_SHORT):
                nc.tensor.matmul(S0[:, 0:128], wsA, wsA,
                                 start=True, stop=True)
            for _ in range(N_WARM_LONG):
                nc.tensor.matmul(S0[:], wsA, wsB,
                                 start=True, stop=True)

            # ---- layer 1 matmuls; K=1 bias matmul runs FIRST ----
            nc.tensor.matmul(ps_hr, Dh_sb[0:1, 3 * H + _BRR:3 * H + _BRR + H],
                             ones_sb[0:1, 0:JC], start=True, stop=False)
            xo = 3 * H
            rhs_r = [
                Dh_sb[:, xo + _XLJ:xo + _XLJ + JC],          # x_l[j]
                Dh_sb[:, xo + _XRH + 2:xo + _XRH + 2 + JC],  # x_r[j+1]
                Dh_sb[:, xo + _XRH:xo + _XRH + JC],          # x_r[j-1]
            ]
            for c in range(3):
                nc.tensor.matmul(
                    ps_hr, Dh_sb[:, c * H:(c + 1) * H],
                    rhs_r[c], start=False, stop=(c == 2),
                )

            # hl layer-1 in TWO 256-col groups (separate psum tiles) so
            # each group's celu pipeline starts as soon as ITS matmuls
            # land, overlapping the other group's matmuls.
            xb = 3 * H + _D1XW
            for g, ps_g in ((0, ps_hl0), (1, ps_hl1)):
                i0 = g * 256
                nc.tensor.matmul(ps_g,
                                 Dh_sb[0:1, 3 * H + _BLR:3 * H + _BLR + H],
                                 ones_sb[0:1, 0:256], start=True, stop=False)
                rhs_l = [
                    Dh_sb[:, xb + 1 + i0:xb + 1 + i0 + 256],  # x_l[i]
                    Dh_sb[:, xb + 0 + i0:xb + i0 + 256],      # x_l[i-1]
                    Dh_sb[:, xb + 2 + i0:xb + 2 + i0 + 256],  # x_l[i+1]
                ]
                for c in range(3):
                    nc.tensor.matmul(
                        ps_g, Dh_sb[:, c * H:(c + 1) * H],
                        rhs_l[c], start=False, stop=(c == 2),
                    )

            # ---- hr celu: e (ACT) -> TS min (DVE) -> TT max (DVE) ----
            e_r = pp.tile([128, JC], F32, name="e_r")
            nc.scalar.activation(e_r[:], ps_hr,
                                 mybir.ActivationFunctionType.Exp)
            nc.vector.tensor_scalar(e_r[:], e_r[:], -1.0, 0.0,
                                    mybir.AluOpType.add,
                                    mybir.AluOpType.min)
            nc.vector.tensor_tensor(hrT[:], ps_hr, e_r[:],
                                    mybir.AluOpType.max)

            # ---- hl celu, pipelined per group (separate e_l tiles so
            # group 1's EXP has no coarse-tile dep on group 0's TS/TT)
            hlT = pp.tile([128, N], BF16, name="hlT")
            e_l0 = pp.tile([128, 256], F32, name="e_l0")
            e_l1 = pp.tile([128, 256], F32, name="e_l1")
            for g, (ps_g, e_g) in enumerate(((ps_hl0, e_l0), (ps_hl1, e_l1))):
                i0 = g * 256
                nc.scalar.activation(e_g[:], ps_g,
                                     mybir.ActivationFunctionType.Exp)
                nc.vector.tensor_scalar(e_g[:], e_g[:], -1.0, 0.0,
                                        mybir.AluOpType.add,
                                        mybir.AluOpType.min)
                nc.vector.tensor_tensor(hlT[:, i0:i0 + 256], ps_g,
                                        e_g[:], mybir.AluOpType.max)

            # ---- v2 matmuls: psum [h, (o,j)] per og group ----
            for ps_vo, o0 in ((ps_v0, 0), (ps_v1, 4)):
                for ol in range(4):
                    o = o0 + ol
                    nc.tensor.matmul(
                        ps_vo[:, ol * JC:(ol + 1) * JC],
                        W_sb[:, o * H:(o + 1) * H], hrT[:],
                        start=True, stop=True,
                    )

            # HAM keep-alive: bridge the PE idle window between v2 and
            # main.  lhsT = hrT pins the dependency so the scheduler
            # cannot hoist these before the layer-1/v2 matmuls.
            for _ in range(2):
                nc.tensor.matmul(
                    S0[:, 0:256], hrT, warm_sb[:, 128:384],
                    start=True, stop=True,
                )

            # ---- v2 casts to interleaved bf16 layout (col = j*8+o),
            # one per og group: og0/og1 live in different psum tiles,
            # so cast0 starts as soon as og0's 4 matmuls land.
            v2sb = pp.tile([128, O * H], BF16, name="v2sb")
            v2v = v2sb[:].rearrange("p (j g o) -> p j g o", g=2, o=4)
            nc.scalar.copy(v2v[:, :, 0, :],
                           ps_v0.rearrange("p (o j) -> p j o", o=4))
            nc.scalar.copy(v2v[:, :, 1, :],
                           ps_v1.rearrange("p (o j) -> p j o", o=4))

            # ---- main (transposed): psum[jo-block, i] = v2_c.T @ hlT ----
            # chunk c partition p -> j = 16c + p//8, o = p%8
            main_dst = [
                S0[:], P12[:, 0:512], P12[:, 512:1024],
                P34[:, 0:512], P34[:, 512:1024],
                P56[:, 0:512], P56[:, 512:1024], S7[:],
            ]
            for c in range(8):
                nc.tensor.matmul(
                    main_dst[c], v2sb[:, c * JC:(c + 1) * JC], hlT[:],
                    start=True, stop=True,
                )

            # separate staging tiles per output DMA: out_sb as ONE tile
            # would serialize the evictions via coarse WAW tracking.
            obs = [pp.tile([128, w], BF16, name=f"ob{i}")
                   for i, w in enumerate(_OW)]
            ob_ap = Dh_sb[:, 3 * H + _OBI:3 * H + _OBI + 2].bitcast(F32)

            def evict(eng, src, dst):
                if eng is nc.scalar:
                    nc.scalar.activation(dst, src,
                                         mybir.ActivationFunctionType.Identity,
                                         bias=ob_ap, scale=1.0)
                else:
                    nc.vector.tensor_scalar_add(dst, src, ob_ap)

            # pair evictions chase the main matmuls; each DMA issues
            # right after its eviction.  ACT: e0, e34, e7; DVE: e12, e56.
            evict(nc.scalar, S0[:], obs[0][:])            # e0  (c0)
            nc.sync.dma_start(outs[0][:], obs[0][:])
            evict(nc.vector, P12[:], obs[1][:])           # e12 (c1,c2)
            nc.sync.dma_start(outs[1][:], obs[1][:])
            evict(nc.scalar, P34[:], obs[2][:])           # e34 (c3,c4)
            nc.sync.dma_start(outs[2][:], obs[2][:])
            evict(nc.vector, P56[:], obs[3][:])           # e56 (c5,c6)
            nc.sync.dma_start(outs[3][:], obs[3][:])
            evict(nc.scalar, S7[:], obs[4][:])            # e7  (c7)
            nc.sync.dma_start(outs[4][:], obs[4][:])

    _legalize_waits(nc)
    return nc


def _legalize_waits(nc):
    """walrus's per-instruction HW structs carry at most ONE sync wait.
    Split any instruction with >1 on_wait into same-engine single-wait
    EventSemaphore predecessors (engine executes them in program order)."""
    n = 0
    for bb in nc.main_func.blocks:
        insts = list(bb.instructions)
        out = []
        for ins in insts:
            si = ins.sync_info
            waits = list(si.on_wait) if si and si.on_wait else []
            if len(waits) > 1:
                for w in waits[:-1]:
                    n += 1
                    out.append(mybir.InstEventSemaphore(
                        name=f"wait-split-{n}",
                        opcode="EventSemaphore",
                        engine=ins.engine,
                        ins=[], outs=[],
                        sync_info=mybir.SyncInfo(on_wait=[w], on_update=[]),
                    ))
                si.on_wait = [waits[-1]]
            out.append(ins)
        if n:
            bb.instructions = out
    return nc


_NC_CACHE = None


def _get_nc():
    global _NC_CACHE
    if _NC_CACHE is None:
        _NC_CACHE = build_nc()
    return _NC_CACHE


def _prep_core_inputs(x_l, x_r, fc_l_W, fc_l_b, fc_r_W, fc_r_b, bilinear_W, bilinear_b):
    """Host-side sharding: build the 8 per-core input dicts."""
    import ml_dtypes

    f32 = np.float32
    bf16 = ml_dtypes.bfloat16
    x_l = np.ascontiguousarray(x_l, f32)
    x_r = np.ascontiguousarray(x_r, f32)

    # WT[g, o*H + h] = W[o, h, g]
    WT = np.ascontiguousarray(
        np.asarray(bilinear_W, f32).transpose(2, 0, 1).reshape(128, O * H)
    ).astype(bf16)

    D1w = np.zeros((128, 3 * H), bf16)
    frW = np.asarray(fc_r_W, f32)
    flW = np.asarray(fc_l_W, f32)
    for c in range(3):
        D1w[:NIN, c * H:(c + 1) * H] = frW[:, c * NIN:(c + 1) * NIN].T.astype(bf16)
        D1w[NIN:, c * H:(c + 1) * H] = flW[:, c * NIN:(c + 1) * NIN].T.astype(bf16)

    D1x_c = np.zeros((128, _D1XW), bf16)
    D1x_c[0, _BRR:_BRR + H] = np.asarray(fc_r_b, f32).astype(bf16)
    D1x_c[0, _BLR:_BLR + H] = np.asarray(fc_l_b, f32).astype(bf16)
    obi = np.asarray(bilinear_b, f32)[np.arange(128) % O]  # bb[p%8]
    D1x_c.view(np.uint16)[:, _OBI:_OBI + 2] = obi.reshape(-1, 1).view('<u2')

    # D1b per batch: xlhT rows 64:128, col t = x_l[b, t-1]
    D1bs = []
    for b in range(B):
        D1b = np.zeros((128, _D1BW), bf16)
        D1b[NIN:, 1:1 + N] = x_l[b].T.astype(bf16)
        D1bs.append(D1b)

    in_maps = []
    for core in range(N_CORES):
        b, jg = core // 4, core % 4
        j0 = jg * JC
        D1x = D1x_c.copy()
        D1x[:NIN, _XLJ:_XLJ + JC] = x_l[b, j0:j0 + JC].T.astype(bf16)
        # xrhT: col t = x_r[b, j0-1+t], zero-padded at global edges
        lo = max(j0 - 1, 0)
        hi = min(j0 + JC + 1, N)
        D1x[:NIN, _XRH + lo - (j0 - 1):_XRH + hi - (j0 - 1)] = \
            x_r[b, lo:hi].T.astype(bf16)
        in_maps.append({
            "Dh": np.concatenate([D1w, D1x, D1bs[b]], axis=1),
            "Wt": WT,
        })
    return in_maps


def _run(inputs, trace=False, **kw):
    nc = _get_nc()
    in_maps = _prep_core_inputs(**inputs)
    res = run_bass_kernel_spmd(
        nc, in_maps, core_ids=list(range(N_CORES)), trace=trace, **kw)
    out = np.empty((B, N, N, O), np.float32)
    for core in range(N_CORES):
        b, jg = core // 4, core % 4
        j0 = jg * JC
        # device out: [p = jr*8+o, c*512 + i] -> out[i, 16c+jr, o]
        r = res.results[core]
        arr = np.concatenate(
            [np.asarray(r[f"out{i}"]) for i in range(5)],
            axis=1).astype(np.float32)
        arr = arr.reshape(16, 8, 8, N)          # [jr, o, c, i]
        out[b, :, j0:j0 + JC, :] = \
            arr.transpose(3, 2, 0, 1).reshape(N, JC, O)
    return out, res


def kernel(**inputs):
    out, _ = _run(inputs, trace=False)
    return out


# revision 11
# speedup vs baseline: 1.1097x; 1.0254x over previous
"""Trainium2 Bass kernel for nn_BilinearPairedLayer.

out[b,i,j,o] = celu(zl[b,i] @ fc_l_W^T + fc_l_b) @ W[o] @ celu(zr[b,j] @ fc_r_W^T + fc_r_b) + bb[o]

with context-3 pairing:
  zl = [x_l, shift_fwd(x_l,1), shift_bwd(x_l,1)]   (192 features)
  zr = [x_l, shift_bwd(x_r,1), shift_fwd(x_r,1)]   (faithful torch-source bug: x_l first)

Shapes: B=2, N=512, n_in=64, H=128, n_out=8 -> out [2,512,512,8] f32.

Sharding: 8 cores = (b in {0,1}) x (j-chunk in {0..3} of 128 columns).
Each core computes out[b, :, j0:j0+128, :] (as bf16; host upcasts).

Per-core dataflow (contraction dims pre-transposed onto partitions host-side).

DMA strategy (empirically tuned against NTFF packet/semaphore data):
  - ALL inputs ride the sync HWDGE ring: one merged "hot" DMA first
    (D1w fc-weight overlay | D1x x-slices+biases | D1b xlhT) whose
    first-on-ring semaphore completes fast, then Wt.
  - The scalar ring carries only the ACT table load (pulled early by a
    dummy Exp).
  - Outputs are bf16 in FOUR paired DMAs on the sync ring, each issued
    as soon as its two chunk evictions land, so the 16 shared DMA mover
    engines stream output packets while later main matmuls still run.
    Each DMA writes its own DENSE [128,1024] DRAM tensor (slices of one
    big tensor write 2KB chunks at 8KB stride: ~185 GB/s vs ~310 dense).

Compute schedule:
  0. PE warm-up matmuls on memset tiles fill the ENTIRE input-DMA wait
     (~7.3us -> ~12.5us): the PE_HAM clock gate un-throttles (1.2 ->
     2.4 GHz) only after a ~3.4us fully-busy activity window, so the
     warm stream is sized to keep the PE busy with no gaps until the
     input semaphore lands.  4 short (128-wide) warms start as early as
     possible off a tiny memset; 5 long (512-wide) warms carry to the
     DMA horizon.  2 keep-alive matmuls (lhsT = hrT pins them after v2)
     bridge the v2->main PE gap.
  1. fc biases are accumulated INTO PSUM by K=1 rank-1 matmuls
     (bias-row^T @ ones-row) that run FIRST in each accumulation group:
     celu = EXP (ACT, psum src) -> fused (-1,min 0) TS (DVE) -> max TT
     (DVE, psum operand), writing bf16.  hr first (feeds v2), hl full.
  2. v2[h, j*8+o]: per o: WT_o.T @ hrT -> psum [h, o*128+j]; TWO
     strided casts (one per og group, both on ACT after hl's EXP)
     write the j-major/o-fast INTERLEAVED bf16 layout; cast og0
     overlaps the og1 matmuls.  With this order, main-output partition
     p has o = p%8 for every chunk, so ONE shared [128,1] bias AP
     serves all evictions.
  3. TRANSPOSED main: psum[jo-block, i] = v2_c.T @ hlT, 8 matmuls N=512.
     PSUM tiles are separate (PSUM dependency tracking is coarse per
     tile); main dsts reuse retired tiles so every implicit tile-level
     WAR dep is one the schedule satisfies anyway:
       pairA: main c0/c1, then c7 (emitted AFTER evict0 -> correct WAR)
       pairB: main c2/c3;  vAB (v2 og psum): main c4/c5 (after casts)
       ps_hl: main c6 (after hl celu reads)
  4. Evictions are PER-CHUNK [128,512] fused copy+bias+bf16-cast ops
     alternating ACT (activation-with-bias) and DVE (tensor_scalar),
     ordered to chase the main matmul completions; each pair's DMA is
     issued immediately after its two evictions.

walrus's per-instruction HW structs carry at most ONE sync wait; a post-pass
splits multi-wait instructions into single-wait EventSemaphore predecessors.
"""

import numpy as np

import concourse.bass as bass
import concourse.mybir as mybir
import concourse.tile as tile
from concourse.bass_utils import run_bass_kernel_spmd

F32 = mybir.dt.float32
BF16 = mybir.dt.bfloat16

B = 2
N = 512
NIN = 64
H = 128
O = 8
JC = 128  # j-chunk per core
N_CORES = 8

# D1x packed-column offsets (bf16 elements)
_XLJ = 0              # xljT  [128]   (rows 0:64)
_XRH = 128            # xrhT  [130]   (rows 0:64)
_BRR = 258            # fc_r_b as a row on partition 0  [128]
_BLR = 386            # fc_l_b as a row on partition 0  [128]
_OBI = 514            # out-bias bb[p%8] per partition (f32 bitcast, 2 cols)
_D1XW = 516
_D1BW = 514           # xlhT (rows 64:128)

N_WARM_SHORT = 4      # 128-wide, start off the tiny wsA memset
N_WARM_LONG = 7       # 512-wide, carry PE activity to the DMA horizon


def build_nc():
    nc = bass.Bass("TRN2")

    Dh = nc.dram_tensor("Dh", [128, 3 * H + _D1XW + _D1BW], BF16,
                        kind="ExternalInput")
    Wt = nc.dram_tensor("Wt", [128, O * H], BF16, kind="ExternalInput")
    # separate dense DRAM tensors per output DMA (see docstring);
    # first and last are single chunks so the packet stream starts as
    # early as possible and the tail DMA is short.
    _OW = [512, 1024, 1024, 1024, 512]
    outs = [nc.dram_tensor(f"out{i}", [128, w], BF16,
                           kind="ExternalOutput") for i, w in enumerate(_OW)]

    with tile.TileContext(nc) as tc:
        with (
            tc.tile_pool(name="persist", bufs=1) as pp,
            tc.tile_pool(name="psum", bufs=1, space="PSUM") as psp,
        ):
            Dh_sb = pp.tile([128, 3 * H + _D1XW + _D1BW], BF16, name="Dh_sb")
            W_sb = pp.tile([128, O * H], BF16, name="W_sb")
            warm_sb = pp.tile([128, 640], BF16, name="warm_sb")
            ones_sb = pp.tile([1, N], BF16, name="ones_sb")
            hrT = pp.tile([128, JC], BF16, name="hrT")
            wsA = warm_sb[:, 0:128]
            wsB = warm_sb[:, 128:640]

            # ---- ALL inputs on the sync ring: the hot layer-1 block
            # (incl. D1b) first so it drains at full rate with a fast
            # first-on-ring sem; Wt behind it.
            nc.sync.dma_start(Dh_sb[:], Dh[:])
            nc.sync.dma_start(W_sb[:], Wt[:])

            # ---- warm-tile memsets (short lhs first so warms start
            # ASAP) + early ACT table load via a dummy Exp (dst = hrT
            # cell: only a WAW dep against the much-later celu TT).
            nc.vector.memset(wsA, 0.0)
            nc.vector.memset(wsB, 0.0)
            nc.vector.memset(ones_sb[:], 1.0)
            nc.scalar.activation(hrT[0:1, 0:1], warm_sb[0:1, 0:1],
                                 mybir.ActivationFunctionType.Exp)

            # PSUM map: separate tiles (PSUM dep tracking is coarse per
            # tile, so aliasing views would serialize unrelated ops).
            # Main matmul dsts reuse retired tiles; each reuse's implicit
            # tile-level WAR dep is one that the schedule satisfies anyway.
            ps_hr = psp.tile([128, JC], F32, name="ps_hr")
            ps_hl = psp.tile([128, N], F32, name="ps_hl")
            vAB = psp.tile([128, 1024], F32, name="vAB")
            pairA = psp.tile([128, 1024], F32, name="pairA")
            pairB = psp.tile([128, 1024], F32, name="pairB")
            ps_v0 = vAB[:, 0:512]
            ps_v1 = vAB[:, 512:1024]

            # ---- PE warm-up stream: no gaps until the input DMA lands
            for _ in range(N_WARM_SHORT):
                nc.tensor.matmul(ps_hl[:, 0:128], wsA, wsA,
                                 start=True, stop=True)
            for _ in range(N_WARM_LONG):
                nc.tensor.matmul(ps_hl[:], wsA, wsB,
                                 start=True, stop=True)

            # ---- layer 1 matmuls; K=1 bias matmul runs FIRST ----
            nc.tensor.matmul(ps_hr[:], Dh_sb[0:1, 3 * H + _BRR:3 * H + _BRR + H],
                             ones_sb[0:1, 0:JC], start=True, stop=False)
            xo = 3 * H
            rhs_r = [
                Dh_sb[:, xo + _XLJ:xo + _XLJ + JC],          # x_l[j]
                Dh_sb[:, xo + _XRH + 2:xo + _XRH + 2 + JC],  # x_r[j+1]
                Dh_sb[:, xo + _XRH:xo + _XRH + JC],          # x_r[j-1]
            ]
            for c in range(3):
                nc.tensor.matmul(
                    ps_hr[:], Dh_sb[:, c * H:(c + 1) * H],
                    rhs_r[c], start=False, stop=(c == 2),
                )

            nc.tensor.matmul(ps_hl[:], Dh_sb[0:1, 3 * H + _BLR:3 * H + _BLR + H],
                             ones_sb[0:1, 0:N], start=True, stop=False)
            xb = 3 * H + _D1XW
            rhs_l = [
                Dh_sb[:, xb + 1:xb + 1 + N],    # x_l[i]
                Dh_sb[:, xb + 0:xb + N],        # x_l[i-1] (fwd)
                Dh_sb[:, xb + 2:xb + 2 + N],    # x_l[i+1] (bwd)
            ]
            for c in range(3):
                nc.tensor.matmul(
                    ps_hl[:], Dh_sb[:, c * H:(c + 1) * H],
                    rhs_l[c], start=False, stop=(c == 2),
                )

            # ---- hr celu: e (ACT) -> TS min (DVE) -> TT max (DVE) ----
            e_r = pp.tile([128, JC], F32, name="e_r")
            nc.scalar.activation(e_r[:], ps_hr[:],
                                 mybir.ActivationFunctionType.Exp)
            nc.vector.tensor_scalar(e_r[:], e_r[:], -1.0, 0.0,
                                    mybir.AluOpType.add,
                                    mybir.AluOpType.min)
            nc.vector.tensor_tensor(hrT[:], ps_hr[:], e_r[:],
                                    mybir.AluOpType.max)

            # ---- v2 matmuls: psum [h, (o,j)] per og group ----
            for og, ps_vo in ((0, ps_v0), (1, ps_v1)):
                for ol in range(4):
                    o = og * 4 + ol
                    nc.tensor.matmul(
                        ps_vo[:, ol * JC:(ol + 1) * JC],
                        W_sb[:, o * H:(o + 1) * H], hrT[:],
                        start=True, stop=True,
                    )

            # HAM keep-alive: bridge the PE idle window between v2 and
            # main.  lhsT = hrT pins the dependency so the scheduler
            # cannot hoist these before the layer-1/v2 matmuls.
            for _ in range(2):
                nc.tensor.matmul(
                    pairA[:, 0:256], hrT, warm_sb[:, 128:384],
                    start=True, stop=True,
                )

            # ---- hl celu (full width; splitting into halves loses:
            # the halves share tiles, so coarse per-tile dep tracking
            # serializes EXP-half-1 behind half-0's TS/TT, and the
            # cast gets scheduled between the EXPs) ----
            hlT = pp.tile([128, N], BF16, name="hlT")
            e_l = pp.tile([128, N], F32, name="e_l")
            nc.scalar.activation(e_l[:], ps_hl[:],
                                 mybir.ActivationFunctionType.Exp)
            nc.vector.tensor_scalar(e_l[:], e_l[:], -1.0, 0.0,
                                    mybir.AluOpType.add,
                                    mybir.AluOpType.min)
            nc.vector.tensor_tensor(hlT[:], ps_hl[:],
                                    e_l[:], mybir.AluOpType.max)

            # ---- v2 cast to interleaved bf16 layout (col = j*8+o) as
            # ONE 4-D strided op over both og groups (0.92 col/ns vs
            # 0.75 for two split casts: the og0-overlap a split would
            # buy never materializes because ACT is EXPing hl anyway).
            v2sb = pp.tile([128, O * H], BF16, name="v2sb")
            nc.scalar.copy(
                v2sb[:].rearrange("p (j g o) -> p j g o", g=2, o=4),
                vAB[:].rearrange("p (g o j) -> p j g o", g=2, o=4))

            # ---- main (transposed): psum[jo-block, i] = v2_c.T @ hlT ----
            # chunk c partition p -> j = 16c + p//8, o = p%8
            main_dst = [
                pairA[:, 0:512], pairA[:, 512:1024],
                pairB[:, 0:512], pairB[:, 512:1024],
                vAB[:, 0:512], vAB[:, 512:1024],   # WAR: after og casts
                ps_hl[:],                          # WAR: after hl celu reads
            ]
            for c in range(7):
                nc.tensor.matmul(
                    main_dst[c], v2sb[:, c * JC:(c + 1) * JC], hlT[:],
                    start=True, stop=True,
                )

            out_sb = pp.tile([128, O * N], BF16, name="out_sb")
            ob_ap = Dh_sb[:, 3 * H + _OBI:3 * H + _OBI + 2].bitcast(F32)

            def evict(eng, src, col0):
                dst = out_sb[:, col0:col0 + 512]
                if eng is nc.scalar:
                    nc.scalar.activation(dst, src,
                                         mybir.ActivationFunctionType.Identity,
                                         bias=ob_ap, scale=1.0)
                else:
                    nc.vector.tensor_scalar_add(dst, src, ob_ap)

            # chunk -> psum src (c7 reuses pairA[0:512]; emitted AFTER
            # evict0 so the WAR dep lands correctly)
            evict(nc.scalar, pairA[:, 0:512], 0)          # e0
            nc.sync.dma_start(outs[0][:], out_sb[:, 0:512])
            evict(nc.vector, pairA[:, 512:1024], 512)     # e1
            # c7 emitted AFTER e1: PSUM dep tracking is coarse per
            # tile, so an e1 emitted later would wait c7's write to
            # the pairA tile even though the column ranges differ.
            nc.tensor.matmul(                             # main c7
                pairA[:, 0:512], v2sb[:, 7 * JC:8 * JC], hlT[:],
                start=True, stop=True,
            )
            evict(nc.scalar, pairB[:, 0:512], 1024)       # e2
            nc.sync.dma_start(outs[1][:], out_sb[:, 512:1536])
            evict(nc.vector, pairB[:, 512:1024], 1536)    # e3
            evict(nc.scalar, vAB[:, 0:512], 2048)         # e4
            nc.sync.dma_start(outs[2][:], out_sb[:, 1536:2560])
            evict(nc.vector, vAB[:, 512:1024], 2560)      # e5
            evict(nc.scalar, ps_hl[:], 3072)              # e6
            nc.sync.dma_start(outs[3][:], out_sb[:, 2560:3584])
            evict(nc.scalar, pairA[:, 0:512], 3584)       # e7
            nc.sync.dma_start(outs[4][:], out_sb[:, 3584:4096])

    _legalize_waits(nc)
    return nc


def _legalize_waits(nc):
    """walrus's per-instruction HW structs carry at most ONE sync wait.
    Split any instruction with >1 on_wait into same-engine single-wait
    EventSemaphore predecessors (engine executes them in program order)."""
    n = 0
    for bb in nc.main_func.blocks:
        insts = list(bb.instructions)
        out = []
        for ins in insts:
            si = ins.sync_info
            waits = list(si.on_wait) if si and si.on_wait else []
            if len(waits) > 1:
                for w in waits[:-1]:
                    n += 1
                    out.append(mybir.InstEventSemaphore(
                        name=f"wait-split-{n}",
                        opcode="EventSemaphore",
                        engine=ins.engine,
                        ins=[], outs=[],
                        sync_info=mybir.SyncInfo(on_wait=[w], on_update=[]),
                    ))
                si.on_wait = [waits[-1]]
            out.append(ins)
        if n:
            bb.instructions = out
    return nc


_NC_CACHE = None


def _get_nc():
    global _NC_CACHE
    if _NC_CACHE is None:
        _NC_CACHE = build_nc()
    return _NC_CACHE


def _prep_core_inputs(x_l, x_r, fc_l_W, fc_l_b, fc_r_W, fc_r_b, bilinear_W, bilinear_b):
    """Host-side sharding: build the 8 per-core input dicts."""
    import ml_dtypes

    f32 = np.float32
    bf16 = ml_dtypes.bfloat16
    x_l = np.ascontiguousarray(x_l, f32)
    x_r = np.ascontiguousarray(x_r, f32)

    # WT[g, o*H + h] = W[o, h, g]
    WT = np.ascontiguousarray(
        np.asarray(bilinear_W, f32).transpose(2, 0, 1).reshape(128, O * H)
    ).astype(bf16)

    D1w = np.zeros((128, 3 * H), bf16)
    frW = np.asarray(fc_r_W, f32)
    flW = np.asarray(fc_l_W, f32)
    for c in range(3):
        D1w[:NIN, c * H:(c + 1) * H] = frW[:, c * NIN:(c + 1) * NIN].T.astype(bf16)
        D1w[NIN:, c * H:(c + 1) * H] = flW[:, c * NIN:(c + 1) * NIN].T.astype(bf16)

    D1x_c = np.zeros((128, _D1XW), bf16)
    D1x_c[0, _BRR:_BRR + H] = np.asarray(fc_r_b, f32).astype(bf16)
    D1x_c[0, _BLR:_BLR + H] = np.asarray(fc_l_b, f32).astype(bf16)
    obi = np.asarray(bilinear_b, f32)[np.arange(128) % O]  # bb[p%8]
    D1x_c.view(np.uint16)[:, _OBI:_OBI + 2] = obi.reshape(-1, 1).view('<u2')

    # D1b per batch: xlhT rows 64:128, col t = x_l[b, t-1]
    D1bs = []
    for b in range(B):
        D1b = np.zeros((128, _D1BW), bf16)
        D1b[NIN:, 1:1 + N] = x_l[b].T.astype(bf16)
        D1bs.append(D1b)

    in_maps = []
    for core in range(N_CORES):
        b, jg = core // 4, core % 4
        j0 = jg * JC
        D1x = D1x_c.copy()
        D1x[:NIN, _XLJ:_XLJ + JC] = x_l[b, j0:j0 + JC].T.astype(bf16)
        # xrhT: col t = x_r[b, j0-1+t], zero-padded at global edges
        lo = max(j0 - 1, 0)
        hi = min(j0 + JC + 1, N)
        D1x[:NIN, _XRH + lo - (j0 - 1):_XRH + hi - (j0 - 1)] = \
            x_r[b, lo:hi].T.astype(bf16)
        in_maps.append({
            "Dh": np.concatenate([D1w, D1x, D1bs[b]], axis=1),
            "Wt": WT,
        })
    return in_maps


def _run(inputs, trace=False, **kw):
    nc = _get_nc()
    in_maps = _prep_core_inputs(**inputs)
    res = run_bass_kernel_spmd(
        nc, in_maps, core_ids=list(range(N_CORES)), trace=trace, **kw)
    out = np.empty((B, N, N, O), np.float32)
    for core in range(N_CORES):
        b, jg = core // 4, core % 4
        j0 = jg * JC
        # device out: [p = jr*8+o, c*512 + i] -> out[i, 16c+jr, o]
        r = res.results[core]
        arr = np.concatenate(
            [np.asarray(r[f"out{i}"]) for i in range(5)],
            axis=1).astype(np.float32)
        arr = arr.reshape(16, 8, 8, N)          # [jr, o, c, i]
        out[b, :, j0:j0 + JC, :] = \
            arr.transpose(3, 2, 0, 1).reshape(N, JC, O)
    return out, res


def kernel(**inputs):
    out, _ = _run(inputs, trace=False)
    return out
